# revision 17
# baseline (speedup 1.0000x reference)
"""Trainium2 Bass kernel for nn_LpmpGRLayer (Sobel-magnitude avg-pool stats + tiny MLP).

Pipeline per (image, channel) plane [512, 512]:
  gx = sobel_x(x), gy = sobel_y(x)           (depthwise 3x3, zero-pad SAME)
  mag = sqrt(gx^2 + gy^2 + 1e-6)
  P   = avg_pool3x3(mag, stride 1, pad 1, count_include_pad)
  feats = [mean(P), std_unbiased(P)] per channel -> [B, 6]
  y = relu(feats @ w1.T + b1) @ w2.T + b2    -> [B, 64]

Sharding: pure data parallel, 4 images (12 planes) per core across 8 cores.

Per-core implementation (all shapes hardcoded):
  - planes laid out rows-on-partitions: [128 part, 4 tiles, 512 cols]
  - vertical 3-tap convs (smooth/diff/box) on TensorE as 128x128 band-matrix
    matmuls; cross-tile halo rows via single-nonzero-element 128x128 matmuls
    on the adjacent tiles, accumulating into the same PSUM bank
  - horizontal 3-tap combines + squares on VectorE/ScalarE/GpSimd in bf16
  - Sum(P) comes free from row-sums of the horizontal box stage (vertical box
    with zero-pad weights rows by 3 except plane-edge rows by 2)
  - Sum(P^2) from ScalarE Square with accumulate on the PSUM box output
  - final stats contraction over partitions via small matmuls, tiny MLP on PE
"""

import threading
from contextlib import ExitStack

import numpy as np
import ml_dtypes

import concourse.bass as bass
import concourse.bacc as bacc
import concourse.tile as tile
from concourse import mybir
from concourse import bass_utils

F32 = mybir.dt.float32
BF16 = mybir.dt.bfloat16
AF = mybir.ActivationFunctionType
OP = mybir.AluOpType
X_AXIS = mybir.AxisListType.X

N_CORES = 8
B, C, H, W = 32, 3, 512, 512
IMGS = B // N_CORES          # images per core
PLANES = IMGS * C            # planes per core (ch-major: pl = ch*IMGS + img)
NT = H // 128                # row tiles per plane
NPIX = H * W
EPS = 1e-6

BF = ml_dtypes.bfloat16


def _band_consts():
    """Band matrices in lhsT layout (out = lhsT.T @ rhs) plus halo matrices."""
    n = 128
    i = np.arange(n)
    # vertical smooth [1,2,1]: u[r] = x[r-1] + 2 x[r] + x[r+1]
    A_s = np.zeros((n, n), np.float32)
    A_s[i, i] = 2.0
    A_s[i[1:], i[1:] - 1] = 1.0
    A_s[i[:-1], i[:-1] + 1] = 1.0
    # vertical diff [1,0,-1]: v[r] = x[r-1] - x[r+1]
    A_d = np.zeros((n, n), np.float32)
    A_d[i[1:], i[1:] - 1] = 1.0
    A_d[i[:-1], i[:-1] + 1] = -1.0
    # vertical box [1,1,1]
    A_b = np.zeros((n, n), np.float32)
    A_b[i, i] = 1.0
    A_b[i[1:], i[1:] - 1] = 1.0
    A_b[i[:-1], i[:-1] + 1] = 1.0

    def top(k_m1):
        # out[0] += k_m1 * rhs[127]   (rhs = tile above)
        h = np.zeros((n, n), np.float32)
        h[n - 1, 0] = k_m1
        return h.astype(BF)

    def bot(k_p1):
        # out[127] += k_p1 * rhs[0]   (rhs = tile below)
        h = np.zeros((n, n), np.float32)
        h[0, n - 1] = k_p1
        return h.astype(BF)

    e0 = np.zeros((n, 1), np.float32)
    e0[0, 0] = 1.0
    e127 = np.zeros((n, 1), np.float32)
    e127[n - 1, 0] = 1.0

    return {
        "sy": A_s.T.astype(BF), "syt": top(1.0), "syb": bot(1.0),
        "dy": A_d.T.astype(BF), "dyt": top(1.0), "dyb": bot(-1.0),
        "bx": A_b.T.astype(BF), "bxt": top(1.0), "bxb": bot(1.0),
        "e0": e0, "e127": e127,
    }


_CONST_SPECS = [
    ("sy", BF16, [128, 128]), ("syt", BF16, [128, 128]), ("syb", BF16, [128, 128]),
    ("dy", BF16, [128, 128]), ("dyt", BF16, [128, 128]), ("dyb", BF16, [128, 128]),
    ("bx", BF16, [128, 128]), ("bxt", BF16, [128, 128]), ("bxb", BF16, [128, 128]),
    ("e0", F32, [128, 1]), ("e127", F32, [128, 1]),
    ("w1T", F32, [35, 32]), ("b1c", F32, [32, 1]),
    ("w2T", F32, [32, 64]), ("b2c", F32, [64, 1]),
]


def _build_bass():
    nc = bacc.Bacc("TRN2", target_bir_lowering=False, debug=False)

    xs = nc.dram_tensor("xs", [IMGS, C, H, W], F32, kind="ExternalInput")
    cin = {
        name: nc.dram_tensor(name, shape, dt_, kind="ExternalInput")
        for name, dt_, shape in _CONST_SPECS
    }
    scratch = nc.dram_tensor("scratch", [12, 2], F32)
    y = nc.dram_tensor("y", [IMGS, 64], F32, kind="ExternalOutput")

    with tile.TileContext(nc) as tc, ExitStack() as ctx:
        _body(ctx, tc, xs.ap(), y.ap(), scratch.ap(), cin)
    nc.compile()
    return nc


def _body(ctx, tc, xs, y, scratch, cin):
    nc = tc.nc

    singles = ctx.enter_context(tc.tile_pool(name="singles", bufs=1))
    xin = ctx.enter_context(tc.tile_pool(name="xin", bufs=2))
    xhp = ctx.enter_context(tc.tile_pool(name="xh", bufs=2))
    sob = ctx.enter_context(tc.tile_pool(name="sob", bufs=2))
    mbp = ctx.enter_context(tc.tile_pool(name="mb", bufs=2))
    bhp = ctx.enter_context(tc.tile_pool(name="bh", bufs=2))
    scrp = ctx.enter_context(tc.tile_pool(name="scr", bufs=4))
    psA = ctx.enter_context(tc.tile_pool(name="psA", bufs=1, space="PSUM"))
    psB = ctx.enter_context(tc.tile_pool(name="psB", bufs=2, space="PSUM"))

    csb = {}
    for name, dt_, shape in _CONST_SPECS:
        t = singles.tile(shape, dt_, tag=f"c_{name}")
        nc.sync.dma_start(t[:], cin[name].ap())
        csb[name] = t

    eps_sb = singles.tile([128, 1], F32, tag="eps")
    nc.vector.memset(eps_sb[:], EPS)
    zero_sb = singles.tile([128, 1], F32, tag="zero")
    nc.vector.memset(zero_sb[:], 0.0)
    ones_sb = singles.tile([128, 1], F32, tag="ones")
    nc.vector.memset(ones_sb[:], 1.0)

    # accumulators: cols [0:48] = rowsum(Bh) per (plane, tile);
    #               cols [48:96] = rowsum(Phat^2) per (plane, tile)
    rs_all = singles.tile([128, 96], F32, tag="rs")

    for pl in range(PLANES):
        ch, img = divmod(pl, IMGS)
        xplane = xs[img, ch]                              # [512, 512] DRAM
        xv = xplane.rearrange("(t p) w -> p t w", p=128)  # [128, 4, 512]

        xf = xin.tile([128, NT, W], F32, tag="xf")
        nc.sync.dma_start(xf[:], xv)
        xh = xhp.tile([128, NT, W], BF16, tag="xh")
        nc.gpsimd.tensor_copy(xh[:], xf[:])

        bh = bhp.tile([128, NT, W], BF16, tag="bh")

        for hf in range(2):  # half-planes: tiles (2hf, 2hf+1)
            t0 = 2 * hf
            u_ps = psA.tile([128, 2, W], F32, tag="u")
            v_ps = psA.tile([128, 2, W], F32, tag="v")
            for ps, main, topm, botm in ((u_ps, "sy", "syt", "syb"),
                                         (v_ps, "dy", "dyt", "dyb")):
                for i in range(2):
                    nc.tensor.matmul(ps[:, i, :], csb[main][:], xh[:, t0 + i, :],
                                     start=True, stop=False)
                for i in range(2):
                    t = t0 + i
                    last = t == NT - 1
                    if t > 0:
                        nc.tensor.matmul(ps[:, i, :], csb[topm][:], xh[:, t - 1, :],
                                         start=False, stop=last)
                    if not last:
                        nc.tensor.matmul(ps[:, i, :], csb[botm][:], xh[:, t + 1, :],
                                         start=False, stop=True)

            # PSUM -> SBUF copies with one zero guard col each side
            # (only one PSUM operand allowed per instruction)
            ub = sob.tile([128, 2, W + 2], BF16, tag="ub")
            nc.vector.memset(ub[:, :, 0:1], 0.0)
            nc.vector.memset(ub[:, :, W + 1:W + 2], 0.0)
            nc.vector.tensor_copy(ub[:, :, 1:W + 1], u_ps[:])
            vb = sob.tile([128, 2, W + 2], BF16, tag="vb")
            nc.vector.memset(vb[:, :, 0:1], 0.0)
            nc.vector.memset(vb[:, :, W + 1:W + 2], 0.0)
            nc.scalar.activation(vb[:, :, 1:W + 1], v_ps[:], AF.Copy)

            # gx[j] = u[j-1] - u[j+1]   (guard cols give zero pad for free)
            gx = sob.tile([128, 2, W], BF16, tag="gx")
            nc.vector.tensor_tensor(gx[:], ub[:, :, 0:W], ub[:, :, 2:W + 2],
                                    op=OP.subtract)
            # gy[j] = v[j-1] + 2 v[j] + v[j+1] = (v[j-1] + v[j+1]) + 2 v[j]
            s2 = sob.tile([128, 2, W], BF16, tag="s2")
            nc.vector.tensor_tensor(s2[:], vb[:, :, 0:W], vb[:, :, 2:W + 2],
                                    op=OP.add)
            gy = sob.tile([128, 2, W], BF16, tag="gy")
            nc.vector.scalar_tensor_tensor(gy[:], vb[:, :, 1:W + 1], 2.0, s2[:],
                                           op0=OP.mult, op1=OP.add)

            # mag
            q = sob.tile([128, 2, W], BF16, tag="q")
            nc.gpsimd.tensor_tensor(q[:], gx[:], gx[:], op=OP.mult)
            r = sob.tile([128, 2, W], BF16, tag="r")
            nc.scalar.activation(r[:], gy[:], AF.Square, bias=zero_sb[:])
            m2 = sob.tile([128, 2, W], BF16, tag="m2")
            nc.vector.tensor_tensor(m2[:], q[:], r[:], op=OP.add)
            # mg = mag with one zero guard col each side
            mg = mbp.tile([128, 2, W + 2], BF16, tag="mg")
            nc.vector.memset(mg[:, :, 0:1], 0.0)
            nc.vector.memset(mg[:, :, W + 1:W + 2], 0.0)
            nc.scalar.activation(mg[:, :, 1:W + 1], m2[:], AF.Sqrt, bias=eps_sb[:])

            # horizontal box: bh = m[j-1] + m[j] + m[j+1]; accum -> row sums
            wv = sob.tile([128, 2, W], BF16, tag="wv")
            nc.vector.tensor_tensor(wv[:], mg[:, :, 0:W], mg[:, :, 2:W + 2], op=OP.add)
            for i in range(2):
                t = t0 + i
                nc.vector.scalar_tensor_tensor(
                    bh[:, t, :], mg[:, i, 1:W + 1], 1.0, wv[:, i, :],
                    op0=OP.mult, op1=OP.add,
                    accum_out=rs_all[:, pl * 4 + t:pl * 4 + t + 1])

        # vertical box on PE; Phat = 9 * avgpool(mag)
        for hf in range(2):
            t0 = 2 * hf
            pp = psB.tile([128, 2, W], F32, tag="pp")
            for i in range(2):
                nc.tensor.matmul(pp[:, i, :], csb["bx"][:], bh[:, t0 + i, :],
                                 start=True, stop=False)
            for i in range(2):
                t = t0 + i
                last = t == NT - 1
                if t > 0:
                    nc.tensor.matmul(pp[:, i, :], csb["bxt"][:], bh[:, t - 1, :],
                                     start=False, stop=last)
                if not last:
                    nc.tensor.matmul(pp[:, i, :], csb["bxb"][:], bh[:, t + 1, :],
                                     start=False, stop=True)
            for i in range(2):
                t = t0 + i
                scr = scrp.tile([128, W], BF16, tag="p2scr")
                nc.scalar.activation(
                    scr[:], pp[:, i, :], AF.Square, bias=zero_sb[:],
                    accum_out=rs_all[:, 48 + pl * 4 + t:48 + pl * 4 + t + 1])

    # ---- final stats ----
    stats_cat = singles.tile([128, 24], F32, tag="stats_cat")
    rsB_v = rs_all[:, 0:48].rearrange("p (n t) -> p n t", t=4)
    rsP_v = rs_all[:, 48:96].rearrange("p (n t) -> p n t", t=4)
    nc.vector.tensor_reduce(stats_cat[:, 0:12], rsB_v, axis=X_AXIS, op=OP.add)
    nc.vector.tensor_reduce(stats_cat[:, 12:24], rsP_v, axis=X_AXIS, op=OP.add)

    # stA[0:12] = sum_rows rsB per plane; stA[32:44] = sum_rows rsP2
    # stBC[0:12] = rsB at plane row 0;    stBC[32:44] = rsB at plane row 511
    stA = psB.tile([44, 1], F32, tag="pp")
    stBC = psB.tile([44, 1], F32, tag="pp")
    nc.tensor.matmul(stA[0:12, :], stats_cat[:, 0:12], ones_sb[:],
                     start=True, stop=True)
    nc.tensor.matmul(stA[32:44, :], stats_cat[:, 12:24], ones_sb[:],
                     start=True, stop=True)
    nc.tensor.matmul(stBC[0:12, :], rsB_v[:, :, 0], csb["e0"][:],
                     start=True, stop=True)
    nc.tensor.matmul(stBC[32:44, :], rsB_v[:, :, 3], csb["e127"][:],
                     start=True, stop=True)

    # stage the four 12-vectors into SBUF at base partition 0
    stg = singles.tile([12, 4], F32, tag="stg")   # [S_B, S_P2, r0, r511]
    nc.scalar.activation(stg[:, 0:1], stA[0:12, :], AF.Copy)
    nc.scalar.activation(stg[:, 1:2], stA[32:44, :], AF.Copy)
    nc.scalar.activation(stg[:, 2:3], stBC[0:12, :], AF.Copy)
    nc.scalar.activation(stg[:, 3:4], stBC[32:44, :], AF.Copy)

    # SigP = 3*S - r0 - r511 ;  P = Phat/9
    small = singles.tile([12, 8], F32, tag="small")
    t1 = small[:, 0:1]
    sigP = small[:, 1:2]
    A2 = small[:, 2:3]
    Bc2 = small[:, 3:4]
    var = small[:, 4:5]
    ms = small[:, 5:7]   # [mean, std] adjacent for the scratch DMA
    nc.vector.tensor_tensor(t1, stg[:, 2:3], stg[:, 3:4], op=OP.add)
    nc.vector.scalar_tensor_tensor(sigP, stg[:, 0:1], 3.0, t1,
                                   op0=OP.mult, op1=OP.subtract)
    n = float(NPIX)
    c1 = 1.0 / (9.0 * n)
    c2 = 1.0 / (81.0 * (n - 1.0))
    c3 = 1.0 / (81.0 * n * (n - 1.0))
    nc.vector.tensor_tensor(A2, sigP, sigP, op=OP.mult)
    nc.vector.tensor_scalar_mul(Bc2, stg[:, 1:2], c2)
    nc.vector.scalar_tensor_tensor(var, A2, -c3, Bc2, op0=OP.mult, op1=OP.add)
    nc.vector.tensor_scalar_mul(ms[:, 0:1], sigP, c1)
    nc.scalar.activation(ms[:, 1:2], var, AF.Sqrt, bias=zero_sb[0:12, :])

    # feats via DRAM scratch: means -> partitions 0..2, stds -> partitions 32..34
    # (w1T is permuted+zero-padded to [35,32] to match; see make_in_maps)
    nc.sync.dma_start(scratch, ms)
    featsG = singles.tile([35, IMGS], F32, tag="featsG")
    nc.vector.memset(featsG[:], 0.0)
    sc = scratch.rearrange("(c i) s -> s c i", i=IMGS)
    nc.sync.dma_start(featsG[0:3, :], sc[0])
    nc.sync.dma_start(featsG[32:35, :], sc[1])

    h_ps = psB.tile([32, IMGS], F32, tag="pp")
    nc.tensor.matmul(h_ps[:], csb["w1T"][:], featsG[:], start=True, stop=True)
    hT = singles.tile([32, IMGS], F32, tag="hT")
    nc.scalar.activation(hT[:], h_ps[:], AF.Relu, bias=csb["b1c"][:])

    y_ps = psB.tile([64, IMGS], F32, tag="pp")
    nc.tensor.matmul(y_ps[:], csb["w2T"][:], hT[:], start=True, stop=True)
    y_sb = singles.tile([64, IMGS], F32, tag="y_sb")
    nc.scalar.activation(y_sb[:], y_ps[:], AF.Identity, bias=csb["b2c"][:])

    nc.sync.dma_start(y.rearrange("i f -> f i"), y_sb[:])


_lock = threading.Lock()
_cached_nc = None


def _get_nc():
    global _cached_nc
    with _lock:
        if _cached_nc is None:
            _cached_nc = _build_bass()
        return _cached_nc


def make_in_maps(x, w1, b1, w2, b2):
    x = np.ascontiguousarray(np.asarray(x, np.float32))
    w1 = np.asarray(w1, np.float32)
    b1 = np.asarray(b1, np.float32)
    w2 = np.asarray(w2, np.float32)
    b2 = np.asarray(b2, np.float32)
    # means feed partitions 0..2, stds partitions 32..34 (see _body)
    w1Tp = np.zeros((35, 32), np.float32)
    w1Tp[0:3, :] = w1.T[[0, 2, 4], :]
    w1Tp[32:35, :] = w1.T[[1, 3, 5], :]
    shared = dict(
        _band_consts(),
        w1T=w1Tp,
        b1c=np.ascontiguousarray(b1[:, None]),
        w2T=np.ascontiguousarray(w2.T),
        b2c=np.ascontiguousarray(b2[:, None]),
    )
    in_maps = []
    for core in range(N_CORES):
        m = dict(shared)
        m["xs"] = np.ascontiguousarray(x[core * IMGS:(core + 1) * IMGS])
        in_maps.append(m)
    return in_maps


def kernel(x, w1, b1, w2, b2):
    nc = _get_nc()
    in_maps = make_in_maps(x, w1, b1, w2, b2)
    res = bass_utils.run_bass_kernel_spmd(nc, in_maps, core_ids=list(range(N_CORES)))
    out = np.concatenate([r["y"] for r in res.results], axis=0)
    return out.astype(np.float32)
